# revision 2
# baseline (speedup 1.0000x reference)
"""HardTripletLoss (non-hardest branch) on 8 TRN2 NeuronCores.

Math:  loss = mean_{i!=j} relu(d_pos[i] - pdist[i,j] + margin)
  pdist[i,j] = ||x_i||^2 + ||y_j||^2 - 2 x_i.y_j ,  d_pos = diag(pdist)
  =>  relu(G[i,j] + a[i] - b[j])  with  G = 2 x y^T,
      a[i] = ||y_i||^2 - 2 x_i.y_i + margin,  b[j] = ||y_j||^2.
Diagonal (i==j) evaluates to exactly relu(margin) = margin, so we compute the
full unmasked sum and subtract N*margin on the host.

Sharding: x rows split across 8 cores (data parallel), y replicated.
Per core: bf16 matmul G-tiles into PSUM; epilogue split between
 - DVE:  sum_j max(G+a, b) per row (fused scalar_tensor_tensor w/ accum),
         then the known Sum_b is subtracted on the host
           (uses relu(z-b) = max(z, b) - b),
 - ACT:  PE folds -b into PSUM via a K=2 ones x [b_hi;b_lo] matmul (bf16
         hi/lo split keeps b exact to ~1e-3), then activation(Relu, bias=a)
         with free-dim accumulate.
Row-partial sums land in a [128, 64] tile per core; host reduces in f64.
"""

import sys

if "/opt/trn_rl_repo" not in sys.path:
    sys.path.insert(0, "/opt/trn_rl_repo")

import numpy as np

N, D = 8192, 128
NCORES = 8
SH = N // NCORES          # 1024 x-rows per core
MT = SH // 128            # 8 m-tiles (128 rows each)
NT2 = N // 1024           # 8 double-tiles (1024 cols each)
NYT = N // 128            # 64 y row-tiles
MARGIN = 0.2
# double-tile (m,n) handled by ACT when (m*NT2+n) % ACT_MOD == 0, else DVE
ACT_MOD = 2

_cache = {}


def _build():
    import concourse.bass as bass
    import concourse.mybir as mybir
    from concourse import bacc
    from concourse.tile import TileContext
    from concourse.bass import ts

    f32 = mybir.dt.float32
    bf16 = mybir.dt.bfloat16
    Alu = mybir.AluOpType
    Act = mybir.ActivationFunctionType

    nc = bacc.Bacc()
    xb = nc.declare_dram_parameter("xb", [SH, D], bf16, isOutput=False)
    yb = nc.declare_dram_parameter("yb", [N, D], bf16, isOutput=False)
    xf = nc.declare_dram_parameter("xf", [SH, D], f32, isOutput=False)
    ylf = nc.declare_dram_parameter("ylf", [SH, D], f32, isOutput=False)
    yf = nc.declare_dram_parameter("yf", [N, D], f32, isOutput=False)
    out_res = nc.declare_dram_parameter("res", [128, MT * NT2], f32, isOutput=True)
    out_b = nc.declare_dram_parameter("bvec", [1, N], f32, isOutput=True)

    s_b = nc.dram_tensor("s_b", [1, N], f32)
    s_hi = nc.dram_tensor("s_hi", [1, N], bf16)
    s_lo = nc.dram_tensor("s_lo", [1, N], bf16)

    yf3 = yf.rearrange("(t p) d -> p t d", p=128)
    xf3 = xf.rearrange("(t p) d -> p t d", p=128)
    ylf3 = ylf.rearrange("(t p) d -> p t d", p=128)

    with TileContext(nc) as tc:
        with (
            tc.tile_pool(name="big", bufs=1) as big,
            tc.tile_pool(name="ld", bufs=4) as ld,
            tc.tile_pool(name="work", bufs=3) as work,
            tc.tile_pool(name="ps", bufs=3, space="PSUM") as ps,
        ):
            yT = big.tile([128, N], bf16)
            xT = big.tile([128, SH], bf16)
            bbs = [
                big.tile([128, 1024], f32, tag=f"bb{n}", name=f"bb{n}")
                for n in range(NT2)
            ]
            rhs2 = big.tile([2, N], bf16)         # [b_hi ; b_lo]
            nones = big.tile([2, 128], bf16)      # -1, fold weights
            yy = big.tile([128, NYT], f32)        # ||y_j||^2, row-major tile layout
            hi = big.tile([128, NYT], bf16)
            hi32 = big.tile([128, NYT], f32)
            lo32 = big.tile([128, NYT], f32)
            lo = big.tile([128, NYT], bf16)
            z2 = big.tile([128, MT], f32)         # 2 x_i.y_i
            yyl = big.tile([128, MT], f32)        # ||y_i||^2, shard rows
            acol = big.tile([128, MT], f32)       # a per m-tile
            res = big.tile([128, MT * NT2], f32)

            # ---- transposed matmul operand loads (bf16, DMA transpose) ----
            for n in range(NT2):
                nc.sync.dma_start_transpose(
                    yT[:, n * 1024 : (n + 1) * 1024],
                    yb[n * 1024 : (n + 1) * 1024, :],
                )
            for m in range(MT):
                nc.sync.dma_start_transpose(xT[:, ts(m, 128)], xb[ts(m, 128), :])
            nc.vector.memset(nones[:], -1.0)

            # ---- row norms of y (split DVE/ACT), from f32 row-major tiles ----
            for t in range(NYT):
                yt = ld.tile([128, D], f32, tag="ld_y")
                nc.gpsimd.dma_start(yt[:], yf3[:, t, :])
                if t % 2 == 0:
                    scr = work.tile([128, D], f32, tag="sq_act")
                    nc.scalar.activation(
                        scr[:], yt[:], Act.Square,
                        accum_out=yy[:, t : t + 1],
                    )
                else:
                    scr = work.tile([128, D], f32, tag="sq_dve")
                    nc.vector.scalar_tensor_tensor(
                        out=scr[:], in0=yt[:], scalar=1.0, in1=yt[:],
                        op0=Alu.mult, op1=Alu.mult,
                        accum_out=yy[:, t : t + 1],
                    )

            # ---- a = yy_shard + margin - 2 x.y_shard ----
            for m in range(MT):
                xt = ld.tile([128, D], f32, tag="ld_x")
                yl = ld.tile([128, D], f32, tag="ld_yl")
                nc.gpsimd.dma_start(xt[:], xf3[:, m, :])
                nc.gpsimd.dma_start(yl[:], ylf3[:, m, :])
                scr = work.tile([128, D], f32, tag="z2_dve")
                nc.vector.scalar_tensor_tensor(
                    out=scr[:], in0=xt[:], scalar=2.0, in1=yl[:],
                    op0=Alu.mult, op1=Alu.mult,
                    accum_out=z2[:, m : m + 1],
                )
                scr2 = work.tile([128, D], f32, tag="yyl_act")
                nc.scalar.activation(
                    scr2[:], yl[:], Act.Square,
                    accum_out=yyl[:, m : m + 1],
                )
            for m in range(MT):
                nc.vector.scalar_tensor_tensor(
                    out=acol[:, m : m + 1], in0=yyl[:, m : m + 1], scalar=MARGIN,
                    in1=z2[:, m : m + 1], op0=Alu.add, op1=Alu.subtract,
                )

            # ---- b hi/lo split (bf16 + residual) ----
            nc.scalar.activation(hi[:], yy[:], Act.Copy)
            nc.scalar.activation(hi32[:], hi[:], Act.Copy)
            nc.vector.scalar_tensor_tensor(
                out=lo32[:], in0=yy[:], scalar=1.0, in1=hi32[:],
                op0=Alu.mult, op1=Alu.subtract,
            )
            nc.scalar.activation(lo[:], lo32[:], Act.Copy)

            # ---- relayout b via DRAM round-trip: (p,t) -> j = t*128+p ----
            # gpsimd (SWDGE) for compute-dependent DMAs
            nc.gpsimd.dma_start(s_b[0, :].rearrange("(t p) -> p t", p=128), yy[:])
            nc.gpsimd.dma_start(s_hi[0, :].rearrange("(t p) -> p t", p=128), hi[:])
            nc.gpsimd.dma_start(s_lo[0, :].rearrange("(t p) -> p t", p=128), lo[:])
            nc.gpsimd.dma_start(out_b[:], s_b[:])
            nc.gpsimd.dma_start(rhs2[0:1, :], s_hi[:])
            nc.gpsimd.dma_start(rhs2[1:2, :], s_lo[:])
            # partition-broadcast b into 8 x [128, 1024]
            for n in range(NT2):
                nc.gpsimd.dma_start(
                    bbs[n][:],
                    s_b[:, n * 1024 : (n + 1) * 1024].broadcast_to([128, 1024]),
                )

            # ---- main: G tiles + fused epilogue ----
            for m in range(MT):
                for n in range(NT2):
                    idx = m * NT2 + n
                    is_act = (idx % ACT_MOD) == 0
                    pt = ps.tile([128, 1024], f32, tag="g")
                    for h in range(2):
                        nc.tensor.matmul(
                            pt[:, h * 512 : (h + 1) * 512],
                            lhsT=xT[:, ts(m, 128)],
                            rhs=yT[:, n * 1024 + h * 512 : n * 1024 + (h + 1) * 512],
                            start=True, stop=not is_act,
                        )
                    if is_act:
                        for h in range(2):
                            nc.tensor.matmul(
                                pt[:, h * 512 : (h + 1) * 512],
                                lhsT=nones[:],
                                rhs=rhs2[:, n * 1024 + h * 512 : n * 1024 + (h + 1) * 512],
                                start=False, stop=True,
                            )
                        scr = work.tile([128, 1024], f32, tag="ep_act")
                        nc.scalar.activation(
                            scr[:], pt[:], Act.Relu,
                            bias=acol[:, m : m + 1],
                            accum_out=res[:, idx : idx + 1],
                        )
                    else:
                        scr = work.tile([128, 1024], f32, tag="ep_dve")
                        nc.vector.scalar_tensor_tensor(
                            out=scr[:], in0=pt[:], scalar=acol[:, m : m + 1],
                            in1=bbs[n][:],
                            op0=Alu.add, op1=Alu.max,
                            accum_out=res[:, idx : idx + 1],
                        )

            nc.gpsimd.dma_start(out_res[:], res[:])

    return nc


def kernel(x: np.ndarray, y: np.ndarray) -> np.ndarray:
    from concourse.bass_utils import run_bass_kernel_spmd
    import ml_dtypes

    x = np.ascontiguousarray(x, dtype=np.float32)
    y = np.ascontiguousarray(y, dtype=np.float32)

    if "nc" not in _cache:
        nc = _build()
        if not nc.is_finalized():
            nc.finalize()
        _cache["nc"] = nc
    nc = _cache["nc"]

    yb = y.astype(ml_dtypes.bfloat16)
    in_maps = []
    for c in range(NCORES):
        sl = slice(c * SH, (c + 1) * SH)
        in_maps.append({
            "xb": (2.0 * x[sl]).astype(ml_dtypes.bfloat16),
            "yb": yb,
            "xf": x[sl],
            "ylf": y[sl],
            "yf": y,
        })

    _cache["in_maps"] = in_maps
    out = run_bass_kernel_spmd(nc, in_maps, list(range(NCORES)))
    results = out.results

    # host reduction (f64)
    total = 0.0
    for c in range(NCORES):
        total += np.asarray(results[c]["res"], dtype=np.float64).sum()
    b_dev = np.asarray(results[0]["bvec"], dtype=np.float64).reshape(N)
    # subtract Sum_b for every DVE tile (max-trick correction)
    bsum_tile = b_dev.reshape(NT2, 1024).sum(axis=1)
    for m in range(MT):
        for n in range(NT2):
            if (m * NT2 + n) % ACT_MOD != 0:
                total -= NCORES * 128.0 * bsum_tile[n]
    total -= float(N) * float(np.float32(MARGIN))
    return np.float32(total / (float(N) * float(N)))



# revision 4
# speedup vs baseline: 2.6528x; 2.6528x over previous
"""HardTripletLoss (non-hardest branch) on 8 TRN2 NeuronCores — v2.

Math:  loss = mean_{i!=j} relu(d_pos[i] - pdist[i,j] + margin)
  pdist[i,j] = ||x_i||^2 + ||y_j||^2 - 2 x_i.y_j ,  d_pos = diag(pdist)
  =>  term(i,j) = relu(G[i,j] + a[i] - b[j])  with  G = 2 x y^T,
      a[i] = ||y_i||^2 - 2 x_i.y_i + margin,  b[j] = bf16(||y_j||^2).
Diagonal evaluates to ~relu(margin) = margin; host subtracts N*margin.

a and b are O(N*D) host-side precompute (0.01% of FLOPs); the O(N^2*D)
matmul + O(N^2) relu/reduce run on device.

Sharding: x rows split across 8 cores (data parallel), y replicated.
Per core: bf16 G tiles [128, UNIT_W] into PSUM; epilogue split between
 - DVE:  sum_j max(G+a, b) via fused scalar_tensor_tensor w/ accumulate
         (relu(z-b) = max(z,b) - b; host subtracts the known sum of b),
 - ACT:  PE folds -b into PSUM via K=1 matmuls against the b row vector,
         then activation(Relu, bias=a) with free-dim accumulate.
Row-partial sums land in res [128, NU] per core; host reduces in f64.
"""

import sys

if "/opt/trn_rl_repo" not in sys.path:
    sys.path.insert(0, "/opt/trn_rl_repo")

import numpy as np

N, D = 8192, 128
NCORES = 8
SH = N // NCORES          # 1024 x-rows per core
MT = SH // 128            # 8 m-tiles
MARGIN = 0.2

UNIT_W = 2048             # epilogue tile width (psum: UNIT_W/512 banks)
NH = N // UNIT_W          # column blocks per row of m-tiles
NU = MT * NH              # total units per core
PS_BUFS = 8 * 512 // UNIT_W  # use all 8 psum banks


def _is_act(m, nh):
    # engine assignment: ACT (fold path) vs DVE (max-trick)
    return (m + nh) % 2 == 0


_cache = {}


def _build():
    import concourse.mybir as mybir
    from concourse import bacc
    from concourse.tile import TileContext

    f32 = mybir.dt.float32
    bf16 = mybir.dt.bfloat16
    Alu = mybir.AluOpType
    Act = mybir.ActivationFunctionType

    nc = bacc.Bacc()
    xb = nc.declare_dram_parameter("xb", [SH, D], bf16, isOutput=False)   # 2*x shard
    yb = nc.declare_dram_parameter("yb", [N, D], bf16, isOutput=False)    # y full
    bv = nc.declare_dram_parameter("bv", [1, N], bf16, isOutput=False)    # b row
    av = nc.declare_dram_parameter("av", [128, MT], f32, isOutput=False)  # a cols
    out_res = nc.declare_dram_parameter("res", [128, NU], f32, isOutput=True)

    with TileContext(nc) as tc:
        with (
            tc.tile_pool(name="big", bufs=1) as big,
            tc.tile_pool(name="epd", bufs=3) as epd,
            tc.tile_pool(name="epa", bufs=3) as epa,
            tc.tile_pool(name="ps", bufs=PS_BUFS, space="PSUM") as ps,
        ):
            yT = big.tile([128, N], bf16)            # y^T  [d, j]
            xT = big.tile([128, SH], bf16)           # (2x)^T [d, i]
            bb = big.tile([128, N], bf16)            # b broadcast to all partitions
            brow = big.tile([1, N], bf16)            # b as a single row
            acol = big.tile([128, MT], f32)          # a per m-tile column
            negones = big.tile([1, 128], bf16)
            res = big.tile([128, NU], f32)

            nc.vector.memset(negones[:], -1.0)

            # ---- loads ----
            nc.sync.dma_start(acol[:], av[:, :])
            nc.sync.dma_start(brow[:], bv[:, :])
            nc.sync.dma_start_transpose(xT[:], xb[:, :])
            for n in range(8):
                nc.sync.dma_start(
                    bb[:, n * 1024 : (n + 1) * 1024],
                    bv[0:1, n * 1024 : (n + 1) * 1024].broadcast_to([128, 1024]),
                )
                nc.sync.dma_start_transpose(
                    yT[:, n * 1024 : (n + 1) * 1024],
                    yb[n * 1024 : (n + 1) * 1024, :],
                )

            # ---- main: G tiles + fused epilogue ----
            HW = UNIT_W // 512  # matmuls per unit
            for nh in range(NH):
                for m in range(MT):
                    u = nh * MT + m
                    col = m * NH + nh  # res column (m-major for host)
                    is_act = _is_act(m, nh)
                    pt = ps.tile([128, UNIT_W], f32, tag="g")
                    for h in range(HW):
                        nc.tensor.matmul(
                            pt[:, h * 512 : (h + 1) * 512],
                            lhsT=xT[:, m * 128 : (m + 1) * 128],
                            rhs=yT[:, nh * UNIT_W + h * 512 : nh * UNIT_W + (h + 1) * 512],
                            start=True, stop=not is_act,
                        )
                    if is_act:
                        for h in range(HW):
                            nc.tensor.matmul(
                                pt[:, h * 512 : (h + 1) * 512],
                                lhsT=negones[:],
                                rhs=brow[0:1, nh * UNIT_W + h * 512 : nh * UNIT_W + (h + 1) * 512],
                                start=False, stop=True,
                            )
                        scr = epa.tile([128, UNIT_W], bf16, tag="ep_act")
                        nc.scalar.activation(
                            scr[:], pt[:], Act.Relu,
                            bias=acol[:, m : m + 1],
                            accum_out=res[:, col : col + 1],
                        )
                    else:
                        scr = epd.tile([128, UNIT_W], bf16, tag="ep_dve")
                        nc.vector.scalar_tensor_tensor(
                            out=scr[:], in0=pt[:], scalar=acol[:, m : m + 1],
                            in1=bb[:, nh * UNIT_W : (nh + 1) * UNIT_W],
                            op0=Alu.add, op1=Alu.max,
                            accum_out=res[:, col : col + 1],
                        )

            nc.sync.dma_start(out_res[:], res[:])

    return nc


def kernel(x: np.ndarray, y: np.ndarray) -> np.ndarray:
    from concourse.bass_utils import run_bass_kernel_spmd
    import ml_dtypes

    x = np.ascontiguousarray(x, dtype=np.float32)
    y = np.ascontiguousarray(y, dtype=np.float32)

    if "nc" not in _cache:
        nc = _build()
        if not nc.is_finalized():
            nc.finalize()
        _cache["nc"] = nc
    nc = _cache["nc"]

    # host-side O(N*D) prologue: b = bf16(||y_j||^2), a = ||y_i||^2 - 2x_i.y_i + margin
    yy = np.sum(y.astype(np.float64) * y.astype(np.float64), axis=1)
    b16 = yy.astype(np.float32).astype(ml_dtypes.bfloat16)
    z2 = 2.0 * np.sum(x.astype(np.float64) * y.astype(np.float64), axis=1)
    a = (yy - z2 + MARGIN).astype(np.float32)

    yb = y.astype(ml_dtypes.bfloat16)
    bv = b16.reshape(1, N)
    in_maps = []
    for c in range(NCORES):
        sl = slice(c * SH, (c + 1) * SH)
        in_maps.append({
            "xb": (2.0 * x[sl]).astype(ml_dtypes.bfloat16),
            "yb": yb,
            "bv": bv,
            "av": np.ascontiguousarray(a[sl].reshape(MT, 128).T),  # [128, MT]
        })

    _cache["in_maps"] = in_maps
    out = run_bass_kernel_spmd(nc, in_maps, list(range(NCORES)))
    results = out.results

    # host reduction (f64)
    total = 0.0
    for c in range(NCORES):
        total += np.asarray(results[c]["res"], dtype=np.float64).sum()
    bsum_blk = b16.astype(np.float64).reshape(NH, UNIT_W).sum(axis=1)
    # subtract Sum_b for every DVE (max-trick) unit
    for nh in range(NH):
        n_dve = sum(1 for m in range(MT) if not _is_act(m, nh))
        total -= NCORES * n_dve * 128.0 * bsum_blk[nh]
    total -= float(N) * float(np.float32(MARGIN))
    return np.float32(total / (float(N) * float(N)))


# revision 6
# speedup vs baseline: 2.7061x; 1.0201x over previous
"""HardTripletLoss (non-hardest branch) on 8 TRN2 NeuronCores — v2.

Math:  loss = mean_{i!=j} relu(d_pos[i] - pdist[i,j] + margin)
  pdist[i,j] = ||x_i||^2 + ||y_j||^2 - 2 x_i.y_j ,  d_pos = diag(pdist)
  =>  term(i,j) = relu(G[i,j] + a[i] - b[j])  with  G = 2 x y^T,
      a[i] = ||y_i||^2 - 2 x_i.y_i + margin,  b[j] = bf16(||y_j||^2).
Diagonal evaluates to ~relu(margin) = margin; host subtracts N*margin.

a and b are O(N*D) host-side precompute (0.01% of FLOPs); the O(N^2*D)
matmul + O(N^2) relu/reduce run on device.

Sharding: x rows split across 8 cores (data parallel), y replicated.
Per core: bf16 G tiles [128, UNIT_W] into PSUM; epilogue split between
 - DVE:  sum_j max(G+a, b) via fused scalar_tensor_tensor w/ accumulate
         (relu(z-b) = max(z,b) - b; host subtracts the known sum of b),
 - ACT:  PE folds -b into PSUM via K=1 matmuls against the b row vector,
         then activation(Relu, bias=a) with free-dim accumulate.
Row-partial sums land in res [128, NU] per core; host reduces in f64.
"""

import sys

if "/opt/trn_rl_repo" not in sys.path:
    sys.path.insert(0, "/opt/trn_rl_repo")

import numpy as np

N, D = 8192, 128
NCORES = 8
SH = N // NCORES          # 1024 x-rows per core
MT = SH // 128            # 8 m-tiles
MARGIN = 0.2

UNIT_W = 2048             # epilogue tile width (psum: UNIT_W/512 banks)
NH = N // UNIT_W          # column blocks per row of m-tiles
NU = MT * NH              # total units per core
PS_BUFS = 8 * 512 // UNIT_W  # use all 8 psum banks


def _is_act(m, nh):
    # engine assignment: ACT (fold path) vs DVE (max-trick)
    return (m + nh) % 2 == 0


_cache = {}


def _build():
    import concourse.mybir as mybir
    from concourse import bacc
    from concourse.tile import TileContext

    f32 = mybir.dt.float32
    bf16 = mybir.dt.bfloat16
    Alu = mybir.AluOpType
    Act = mybir.ActivationFunctionType

    nc = bacc.Bacc()
    xb = nc.declare_dram_parameter("xb", [SH, D], bf16, isOutput=False)   # 2*x shard
    yb = nc.declare_dram_parameter("yb", [N, D], bf16, isOutput=False)    # y full
    bv = nc.declare_dram_parameter("bv", [1, N], bf16, isOutput=False)    # b row
    av = nc.declare_dram_parameter("av", [128, MT], f32, isOutput=False)  # a cols
    out_res = nc.declare_dram_parameter("res", [128, NU], f32, isOutput=True)

    with TileContext(nc) as tc:
        with (
            tc.tile_pool(name="big", bufs=1) as big,
            tc.tile_pool(name="epd", bufs=3) as epd,
            tc.tile_pool(name="epa", bufs=3) as epa,
            tc.tile_pool(name="ps", bufs=PS_BUFS, space="PSUM") as ps,
        ):
            yT = big.tile([128, N], bf16)            # y^T  [d, j]
            xT = big.tile([128, SH], bf16)           # (2x)^T [d, i]
            bb = big.tile([128, N], bf16)            # b broadcast to all partitions
            brow = big.tile([1, N], bf16)            # b as a single row
            acol = big.tile([128, MT], f32)          # a per m-tile column
            negones = big.tile([1, 128], bf16)
            res = big.tile([128, NU], f32)

            nc.vector.memset(negones[:], -1.0)

            # ---- loads ----
            # sync ring: transpose-mode only (mode transitions serialize the
            # DMA path); copy-mode DMAs go on the scalar ring.
            nc.sync.dma_start_transpose(xT[:], xb[:, :])
            for n in range(8):
                nc.sync.dma_start_transpose(
                    yT[:, n * 1024 : (n + 1) * 1024],
                    yb[n * 1024 : (n + 1) * 1024, :],
                )
            nc.scalar.dma_start(acol[:], av[:, :])
            nc.scalar.dma_start(brow[:], bv[:, :])
            for n in range(8):
                nc.scalar.dma_start(
                    bb[:, n * 1024 : (n + 1) * 1024],
                    bv[0:1, n * 1024 : (n + 1) * 1024].broadcast_to([128, 1024]),
                )

            # ---- main: G tiles + fused epilogue ----
            HW = UNIT_W // 512  # matmuls per unit
            for nh in range(NH):
                for m in range(MT):
                    u = nh * MT + m
                    col = m * NH + nh  # res column (m-major for host)
                    is_act = _is_act(m, nh)
                    pt = ps.tile([128, UNIT_W], f32, tag="g")
                    for h in range(HW):
                        nc.tensor.matmul(
                            pt[:, h * 512 : (h + 1) * 512],
                            lhsT=xT[:, m * 128 : (m + 1) * 128],
                            rhs=yT[:, nh * UNIT_W + h * 512 : nh * UNIT_W + (h + 1) * 512],
                            start=True, stop=not is_act,
                        )
                    if is_act:
                        for h in range(HW):
                            nc.tensor.matmul(
                                pt[:, h * 512 : (h + 1) * 512],
                                lhsT=negones[:],
                                rhs=brow[0:1, nh * UNIT_W + h * 512 : nh * UNIT_W + (h + 1) * 512],
                                start=False, stop=True,
                            )
                        scr = epa.tile([128, UNIT_W], bf16, tag="ep_act")
                        nc.scalar.activation(
                            scr[:], pt[:], Act.Relu,
                            bias=acol[:, m : m + 1],
                            accum_out=res[:, col : col + 1],
                        )
                    else:
                        scr = epd.tile([128, UNIT_W], bf16, tag="ep_dve")
                        nc.vector.scalar_tensor_tensor(
                            out=scr[:], in0=pt[:], scalar=acol[:, m : m + 1],
                            in1=bb[:, nh * UNIT_W : (nh + 1) * UNIT_W],
                            op0=Alu.add, op1=Alu.max,
                            accum_out=res[:, col : col + 1],
                        )

            nc.scalar.dma_start(out_res[:], res[:])

    return nc


def kernel(x: np.ndarray, y: np.ndarray) -> np.ndarray:
    from concourse.bass_utils import run_bass_kernel_spmd
    import ml_dtypes

    x = np.ascontiguousarray(x, dtype=np.float32)
    y = np.ascontiguousarray(y, dtype=np.float32)

    if "nc" not in _cache:
        nc = _build()
        if not nc.is_finalized():
            nc.finalize()
        _cache["nc"] = nc
    nc = _cache["nc"]

    # host-side O(N*D) prologue: b = bf16(||y_j||^2), a = ||y_i||^2 - 2x_i.y_i + margin
    yy = np.sum(y.astype(np.float64) * y.astype(np.float64), axis=1)
    b16 = yy.astype(np.float32).astype(ml_dtypes.bfloat16)
    z2 = 2.0 * np.sum(x.astype(np.float64) * y.astype(np.float64), axis=1)
    a = (yy - z2 + MARGIN).astype(np.float32)

    yb = y.astype(ml_dtypes.bfloat16)
    bv = b16.reshape(1, N)
    in_maps = []
    for c in range(NCORES):
        sl = slice(c * SH, (c + 1) * SH)
        in_maps.append({
            "xb": (2.0 * x[sl]).astype(ml_dtypes.bfloat16),
            "yb": yb,
            "bv": bv,
            "av": np.ascontiguousarray(a[sl].reshape(MT, 128).T),  # [128, MT]
        })

    _cache["in_maps"] = in_maps
    out = run_bass_kernel_spmd(nc, in_maps, list(range(NCORES)))
    results = out.results

    # host reduction (f64)
    total = 0.0
    for c in range(NCORES):
        total += np.asarray(results[c]["res"], dtype=np.float64).sum()
    bsum_blk = b16.astype(np.float64).reshape(NH, UNIT_W).sum(axis=1)
    # subtract Sum_b for every DVE (max-trick) unit
    for nh in range(NH):
        n_dve = sum(1 for m in range(MT) if not _is_act(m, nh))
        total -= NCORES * n_dve * 128.0 * bsum_blk[nh]
    total -= float(N) * float(np.float32(MARGIN))
    return np.float32(total / (float(N) * float(N)))
